# revision 1
# baseline (speedup 1.0000x reference)
"""GRU-style GNN message-passing kernel for Trainium2 (8 NeuronCores, SPMD).

Reference computation (per node b, features 256, 8 neighbors):
    xr = x @ Wir.T + bir
    hr_n = hs_n @ Whr.T + bhr
    r_n = sigmoid(xr + hr_n)
    z = sigmoid(x @ Wiz.T + biz + h_sum @ Whz.T + bhz)
    s = sum_n r_n * hs_n
    n = tanh(x @ Win.T + bin + s @ Whn.T + bhn)
    out = (1 - z) * n + z * h_sum

Strategy: data-parallel over the node dim B=32768 across 8 cores (4096
rows each), batch-chunked 8x512 per core. Everything on-chip runs in
feature-major ("transposed") layout [256 features = 2 partition chunks
of 128, batch free dim], so every linear layer is a natural PE matmul.
Matmuls and the streamed tensors (x, hs) are bf16 (fp32 PSUM
accumulation); h_sum, the z/n gates and the final combine stay fp32 so
the dominant z*h_sum term keeps fp32-level accuracy. Engine placement
per chunk of 512 nodes:
  - PE: all 13 linear-layer matmuls, N=512 moving dim; the shared
    (xr + b_r) term is added into each neighbor's PSUM group via an
    identity matmul; z and n accumulate both their linear terms in PSUM.
  - ACT: sigmoid/tanh (bias per-partition = per-feature); xr+b_r add.
  - DVE: r*hs pair-products and the neighbor add tree, in place in the
    hs tile (bf16 2x mode, contiguous [128,1024+] runs).
  - GPSIMD: h_sum fp32->bf16 cast, final combine out = n + z*(h - n).
"""

import sys
import numpy as np
from contextlib import ExitStack

sys.path.insert(0, "/opt/trn_rl_repo")

import ml_dtypes
import concourse.bacc as bacc
import concourse.tile as tile
from concourse import mybir
from concourse.bass_utils import run_bass_kernel_spmd

F32 = mybir.dt.float32
BF16 = mybir.dt.bfloat16
BF_NP = ml_dtypes.bfloat16

N_NEIGH, B, IN, H = 8, 32768, 256, 256
M = 8                    # cores
BL = B // M              # rows per core (4096)
NCH = 8                  # batch chunks per core
CW = BL // NCH           # chunk width (512)

_cached = None  # compiled program, reused across kernel() calls


def _build():
    nc = bacc.Bacc("TRN2", target_bir_lowering=False, debug=False, num_devices=M)

    xT = nc.dram_tensor("xT", [IN, BL], BF16, kind="ExternalInput").ap()
    hT = nc.dram_tensor("hT", [H, BL], F32, kind="ExternalInput").ap()
    hsT = nc.dram_tensor("hsT", [N_NEIGH, H, BL], BF16, kind="ExternalInput").ap()
    wAP = {}
    for w in ("wir", "whr", "wiz", "whz", "win", "whn"):
        wAP[w] = nc.dram_tensor(w, [256, 256], BF16, kind="ExternalInput").ap()
    ident = nc.dram_tensor("ident", [128, 128], BF16, kind="ExternalInput").ap()
    # bias pack: col f*3+j holds feature-chunk f of (b_r, b_z, b_n)[j]
    biasp = nc.dram_tensor("biasp", [128, 6], F32, kind="ExternalInput").ap()
    outT = nc.dram_tensor("outT", [H, BL], F32, kind="ExternalOutput").ap()

    with tile.TileContext(nc) as tc, ExitStack() as ctx:
        const_pool = ctx.enter_context(tc.tile_pool(name="const", bufs=1))
        x_pool = ctx.enter_context(tc.tile_pool(name="x", bufs=2))
        h_pool = ctx.enter_context(tc.tile_pool(name="h", bufs=2))
        hb_pool = ctx.enter_context(tc.tile_pool(name="hb", bufs=2))
        hs_pool = ctx.enter_context(tc.tile_pool(name="hs", bufs=3))
        xr_pool = ctx.enter_context(tc.tile_pool(name="xr", bufs=2))
        z_pool = ctx.enter_context(tc.tile_pool(name="z", bufs=2))
        s_pool = ctx.enter_context(tc.tile_pool(name="s", bufs=2))
        r_pool = ctx.enter_context(tc.tile_pool(name="r", bufs=2))
        n_pool = ctx.enter_context(tc.tile_pool(name="n", bufs=2))
        d_pool = ctx.enter_context(tc.tile_pool(name="d", bufs=2))
        o_pool = ctx.enter_context(tc.tile_pool(name="o", bufs=2))
        pz_pool = ctx.enter_context(tc.tile_pool(name="pz", bufs=2, space="PSUM"))
        pr_pool = ctx.enter_context(tc.tile_pool(name="pr", bufs=2, space="PSUM"))
        pn_pool = ctx.enter_context(tc.tile_pool(name="pn", bufs=2, space="PSUM"))

        # --- constants ---
        wt = {}
        for w in ("wir", "whr", "wiz", "whz", "win", "whn"):
            wt[w] = []
            for k in range(2):
                t = const_pool.tile([128, 256], BF16, tag=f"{w}{k}", name=f"{w}{k}")
                nc.sync.dma_start(out=t[:, :], in_=wAP[w][k * 128:(k + 1) * 128, :])
                wt[w].append(t)
        id_t = const_pool.tile([128, 128], BF16, tag="ident", name="id_t")
        nc.sync.dma_start(out=id_t[:, :], in_=ident[:, :])
        bias_t = const_pool.tile([128, 6], F32, tag="biasp", name="bias_t")
        nc.sync.dma_start(out=bias_t[:, :], in_=biasp[:, :])

        def fcols(t, f):
            return t[:, f * 128:(f + 1) * 128]

        for c in range(NCH):
            sl = slice(c * CW, (c + 1) * CW)

            # x.T as one [128, 1024] bf16 tile, f-chunk halves (one 3D DMA)
            xt = x_pool.tile([128, 2 * CW], BF16, tag="x", name=f"x_{c}")
            nc.sync.dma_start(
                out=xt[:, :].rearrange("p (f b) -> p f b", f=2),
                in_=xT[:, sl].rearrange("(f p) b -> p f b", f=2))
            # h_sum.T fp32 [128, 1024] + bf16 cast for the Whz matmul
            ht = h_pool.tile([128, 2 * CW], F32, tag="h", name=f"h_{c}")
            nc.sync.dma_start(
                out=ht[:, :].rearrange("p (f b) -> p f b", f=2),
                in_=hT[:, sl].rearrange("(f p) b -> p f b", f=2))
            htb = hb_pool.tile([128, 2 * CW], BF16, tag="hb", name=f"hb_{c}")
            nc.gpsimd.tensor_copy(htb[:, :], ht[:, :])
            # hs.T as one [128, 8192] bf16 tile: (n, f, b) layout, one DMA
            # per neighbor
            hsc = hs_pool.tile([128, 2 * N_NEIGH * CW], BF16, tag="hs",
                               name=f"hs_{c}")
            for n in range(N_NEIGH):
                nc.sync.dma_start(
                    out=hsc[:, n * 2 * CW:(n + 1) * 2 * CW].rearrange(
                        "p (f b) -> p f b", f=2),
                    in_=hsT[n, :, sl].rearrange("(f p) b -> p f b", f=2))

            def hs_n(n):        # [128, 1024] both feature chunks of neighbor n
                return hsc[:, n * 2 * CW:(n + 1) * 2 * CW]

            def hs_slice(n, f):  # [128, 512] matmul operand
                return hsc[:, (n * 2 + f) * CW:(n * 2 + f + 1) * CW]

            # --- z gate: sigmoid(Wiz@x + Whz@h + b_z), PSUM-accumulated ---
            zt = z_pool.tile([128, 2 * CW], F32, tag="z", name=f"z_{c}")
            for f in range(2):
                pz = pz_pool.tile([128, CW], F32, tag="pz", name=f"pz{f}_{c}")
                nc.tensor.matmul(pz[:, :], fcols(wt["wiz"][0], f), xt[:, 0:CW],
                                 start=True, stop=False)
                nc.tensor.matmul(pz[:, :], fcols(wt["wiz"][1], f),
                                 xt[:, CW:2 * CW], start=False, stop=False)
                nc.tensor.matmul(pz[:, :], fcols(wt["whz"][0], f), htb[:, 0:CW],
                                 start=False, stop=False)
                nc.tensor.matmul(pz[:, :], fcols(wt["whz"][1], f),
                                 htb[:, CW:2 * CW], start=False, stop=True)
                nc.scalar.activation(zt[:, f * CW:(f + 1) * CW], pz[:, :],
                                     mybir.ActivationFunctionType.Sigmoid,
                                     bias=bias_t[:, f * 3 + 1:f * 3 + 2])

            # --- xr + b_r, bf16 in SBUF, added into each neighbor's PSUM
            #     group via the identity matmul ---
            xr = xr_pool.tile([128, 2 * CW], BF16, tag="xr", name=f"xr_{c}")
            for f in range(2):
                pxr = pn_pool.tile([128, CW], F32, tag="pn", name=f"pxr{f}_{c}")
                nc.tensor.matmul(pxr[:, :], fcols(wt["wir"][0], f), xt[:, 0:CW],
                                 start=True, stop=False)
                nc.tensor.matmul(pxr[:, :], fcols(wt["wir"][1], f),
                                 xt[:, CW:2 * CW], start=False, stop=True)
                nc.scalar.add(xr[:, f * CW:(f + 1) * CW], pxr[:, :],
                              bias_t[:, f * 3:f * 3 + 1])

            # --- neighbor loop: r = sigmoid(Whr@hs_n + xr + b_r) as bf16;
            #     prod/sum tree runs incrementally as pairs complete ---
            rc = r_pool.tile([128, 2 * N_NEIGH * CW], BF16, tag="r", name=f"r_{c}")
            sc = s_pool.tile([128, 2 * CW], BF16, tag="s", name=f"s_{c}")
            for n in range(N_NEIGH):
                pr = pr_pool.tile([128, 2 * CW], F32, tag="pr", name=f"pr{n}_{c}")
                for f in range(2):
                    ph = pr[:, f * CW:(f + 1) * CW]
                    nc.tensor.matmul(ph, fcols(wt["whr"][0], f), hs_slice(n, 0),
                                     start=True, stop=False)
                    nc.tensor.matmul(ph, fcols(wt["whr"][1], f), hs_slice(n, 1),
                                     start=False, stop=False)
                    nc.tensor.matmul(ph, id_t[:, :], xr[:, f * CW:(f + 1) * CW],
                                     start=False, stop=True)
                nc.scalar.activation(rc[:, n * 2 * CW:(n + 1) * 2 * CW], pr[:, :],
                                     mybir.ActivationFunctionType.Sigmoid)
                if n % 2 == 1:
                    # prod for the (n-1, n) pair: [128, 2048] in place in hsc
                    pair = hsc[:, (n - 1) * 2 * CW:(n + 1) * 2 * CW]
                    nc.vector.tensor_mul(
                        pair, rc[:, (n - 1) * 2 * CW:(n + 1) * 2 * CW], pair)
                    # tree level 1: hsc[n-1] += hsc[n]
                    with nc.allow_low_precision(reason="bf16 neighbor sums"):
                        nc.vector.tensor_add(hs_n(n - 1), hs_n(n - 1), hs_n(n))
            # tree levels 2 + 3 -> s [128, (f, 512)] bf16
            with nc.allow_low_precision(reason="bf16 neighbor sums"):
                nc.vector.tensor_add(hs_n(0), hs_n(0), hs_n(2))
                nc.vector.tensor_add(hs_n(4), hs_n(4), hs_n(6))
                nc.vector.tensor_add(sc[:, :], hs_n(0), hs_n(4))

            # --- n gate: tanh(Win@x + Whn@s + b_n), PSUM-accumulated ---
            nt = n_pool.tile([128, 2 * CW], F32, tag="n", name=f"n_{c}")
            for f in range(2):
                pn = pn_pool.tile([128, CW], F32, tag="pn", name=f"pn{f}_{c}")
                nc.tensor.matmul(pn[:, :], fcols(wt["win"][0], f), xt[:, 0:CW],
                                 start=True, stop=False)
                nc.tensor.matmul(pn[:, :], fcols(wt["win"][1], f),
                                 xt[:, CW:2 * CW], start=False, stop=False)
                nc.tensor.matmul(pn[:, :], fcols(wt["whn"][0], f), sc[:, 0:CW],
                                 start=False, stop=False)
                nc.tensor.matmul(pn[:, :], fcols(wt["whn"][1], f),
                                 sc[:, CW:2 * CW], start=False, stop=True)
                nc.scalar.activation(nt[:, f * CW:(f + 1) * CW], pn[:, :],
                                     mybir.ActivationFunctionType.Tanh,
                                     bias=bias_t[:, f * 3 + 2:f * 3 + 3])

            # --- out = n + z * (h - n) on GPSIMD, [128, 1024] fp32 ---
            dt_ = d_pool.tile([128, 2 * CW], F32, tag="d", name=f"d_{c}")
            nc.gpsimd.tensor_sub(dt_[:, :], ht[:, :], nt[:, :])
            nc.gpsimd.tensor_mul(dt_[:, :], zt[:, :], dt_[:, :])
            ot = o_pool.tile([128, 2 * CW], F32, tag="o", name=f"o_{c}")
            nc.gpsimd.tensor_add(ot[:, :], nt[:, :], dt_[:, :])
            for f in range(2):
                nc.sync.dma_start(out=outT[f * 128:(f + 1) * 128, sl],
                                  in_=ot[:, f * CW:(f + 1) * CW])

    nc.compile()
    return nc


def _prep_inputs(x, h_sum, hs, Wir, bir, Whr, bhr, Wiz, biz, Whz, bhz,
                 Win, bin_, Whn, bhn):
    """Shard + transpose to feature-major per-core input maps."""
    f32 = np.float32
    xT = np.asarray(x, f32).T.astype(BF_NP)                  # [256, B] bf16
    hT = np.ascontiguousarray(np.asarray(h_sum, f32).T)      # [256, B] f32
    hsT = np.asarray(hs, f32).transpose(0, 2, 1).astype(BF_NP)  # [8,256,B] bf16

    w = {
        "wir": np.ascontiguousarray(np.asarray(Wir, f32).T.astype(BF_NP)),
        "whr": np.ascontiguousarray(np.asarray(Whr, f32).T.astype(BF_NP)),
        "wiz": np.ascontiguousarray(np.asarray(Wiz, f32).T.astype(BF_NP)),
        "whz": np.ascontiguousarray(np.asarray(Whz, f32).T.astype(BF_NP)),
        "win": np.ascontiguousarray(np.asarray(Win, f32).T.astype(BF_NP)),
        "whn": np.ascontiguousarray(np.asarray(Whn, f32).T.astype(BF_NP)),
    }
    b_r = np.asarray(bir, f32) + np.asarray(bhr, f32)
    b_z = np.asarray(biz, f32) + np.asarray(bhz, f32)
    b_n = np.asarray(bin_, f32) + np.asarray(bhn, f32)
    biasp = np.empty((128, 6), f32)
    for f in range(2):
        biasp[:, f * 3 + 0] = b_r[f * 128:(f + 1) * 128]
        biasp[:, f * 3 + 1] = b_z[f * 128:(f + 1) * 128]
        biasp[:, f * 3 + 2] = b_n[f * 128:(f + 1) * 128]
    ident = np.eye(128, dtype=f32).astype(BF_NP)

    in_maps = []
    for c in range(M):
        sl = slice(c * BL, (c + 1) * BL)
        m = {
            "xT": np.ascontiguousarray(xT[:, sl]),
            "hT": np.ascontiguousarray(hT[:, sl]),
            "hsT": np.ascontiguousarray(hsT[:, :, sl]),
            "ident": ident,
            "biasp": biasp,
        }
        m.update(w)
        in_maps.append(m)
    return in_maps


def _run(inputs, trace=False, **trace_kwargs):
    global _cached
    if _cached is None:
        _cached = _build()
    nc = _cached
    in_maps = _prep_inputs(**inputs)
    res = run_bass_kernel_spmd(nc, in_maps, list(range(M)), trace=trace,
                               **trace_kwargs)
    out = np.empty((B, H), np.float32)
    for c in range(M):
        out[c * BL:(c + 1) * BL, :] = res.results[c]["outT"].T
    return out, res


def kernel(**inputs):
    return _run(inputs)[0]



# revision 2
# speedup vs baseline: 1.4978x; 1.4978x over previous
"""GRU-style GNN message-passing kernel for Trainium2 (8 NeuronCores, SPMD).

Reference computation (per node b, features 256, 8 neighbors):
    xr = x @ Wir.T + bir
    hr_n = hs_n @ Whr.T + bhr
    r_n = sigmoid(xr + hr_n)
    z = sigmoid(x @ Wiz.T + biz + h_sum @ Whz.T + bhz)
    s = sum_n r_n * hs_n
    n = tanh(x @ Win.T + bin + s @ Whn.T + bhn)
    out = (1 - z) * n + z * h_sum

Strategy: data-parallel over the node dim B=32768 across 8 cores (4096
rows each), batch-chunked 4x1024 per core. Everything on-chip runs in
feature-major layout [256 features = 2 partition chunks of 128, batch
free dim], all tensors bf16 (fp32 PSUM accumulation). Host pre-packs
each chunk tile contiguously in HBM so every DMA moves 4-32KB/partition
lines. Engine placement per chunk of 1024 nodes:
  - PE: all linear-layer matmuls in [128,512] PSUM-bank groups; the
    shared (xr + b_r) term is injected into each neighbor's PSUM group
    via an identity matmul.
  - ACT: sigmoid/tanh (per-feature bias), xr PSUM drain (+b_r).
  - DVE: r*hs products in place in the hs tile and a linear running sum
    (short dependency tail), plus the final combine out = n + z*(h-n),
    all bf16 2x mode.
Chunk gate-work (z, xr) is emitted one chunk ahead, interleaved into the
neighbor loop, so the two 4-bank PSUM slots rotate without stalling PE.
"""

import sys
import numpy as np
from contextlib import ExitStack

sys.path.insert(0, "/opt/trn_rl_repo")

import ml_dtypes
import concourse.bacc as bacc
import concourse.tile as tile
from concourse import mybir
from concourse.bass_utils import run_bass_kernel_spmd

F32 = mybir.dt.float32
BF16 = mybir.dt.bfloat16
BF_NP = ml_dtypes.bfloat16

N_NEIGH, B, IN, H = 8, 32768, 256, 256
M = 8                    # cores
BL = B // M              # rows per core (4096)
NCH = 4                  # batch chunks per core
CW = BL // NCH           # chunk width (1024)

_cached = None  # compiled program, reused across kernel() calls


def _build():
    nc = bacc.Bacc("TRN2", target_bir_lowering=False, debug=False, num_devices=M)

    xC = nc.dram_tensor("xC", [NCH, 128, 2 * CW], BF16, kind="ExternalInput").ap()
    hC = nc.dram_tensor("hC", [NCH, 128, 2 * CW], BF16, kind="ExternalInput").ap()
    hsC = nc.dram_tensor("hsC", [NCH, 128, 2 * N_NEIGH * CW], BF16,
                         kind="ExternalInput").ap()
    wAP = {}
    for w in ("wir", "whr", "wiz", "whz", "win", "whn"):
        wAP[w] = nc.dram_tensor(w, [256, 256], BF16, kind="ExternalInput").ap()
    ident = nc.dram_tensor("ident", [128, 128], BF16, kind="ExternalInput").ap()
    # bias pack: col f*3+j holds feature-chunk f of (b_r, b_z, b_n)[j]
    biasp = nc.dram_tensor("biasp", [128, 6], F32, kind="ExternalInput").ap()
    outS = nc.dram_tensor("outS", [NCH, 128, 2 * CW], BF16,
                          kind="ExternalOutput").ap()

    with tile.TileContext(nc) as tc, ExitStack() as ctx:
        const_pool = ctx.enter_context(tc.tile_pool(name="const", bufs=1))
        x_pool = ctx.enter_context(tc.tile_pool(name="x", bufs=2))
        h_pool = ctx.enter_context(tc.tile_pool(name="h", bufs=2))
        hs_pool = ctx.enter_context(tc.tile_pool(name="hs", bufs=2))
        xr_pool = ctx.enter_context(tc.tile_pool(name="xr", bufs=2))
        z_pool = ctx.enter_context(tc.tile_pool(name="z", bufs=2))
        r_pool = ctx.enter_context(tc.tile_pool(name="r", bufs=4))
        n_pool = ctx.enter_context(tc.tile_pool(name="n", bufs=2))
        d_pool = ctx.enter_context(tc.tile_pool(name="d", bufs=2))
        o_pool = ctx.enter_context(tc.tile_pool(name="o", bufs=2))
        ps_pool = ctx.enter_context(tc.tile_pool(name="ps", bufs=2, space="PSUM"))

        # --- constants ---
        wt = {}
        for w in ("wir", "whr", "wiz", "whz", "win", "whn"):
            wt[w] = []
            for k in range(2):
                t = const_pool.tile([128, 256], BF16, tag=f"{w}{k}", name=f"{w}{k}")
                nc.sync.dma_start(out=t[:, :], in_=wAP[w][k * 128:(k + 1) * 128, :])
                wt[w].append(t)
        id_t = const_pool.tile([128, 128], BF16, tag="ident", name="id_t")
        nc.sync.dma_start(out=id_t[:, :], in_=ident[:, :])
        bias_t = const_pool.tile([128, 6], F32, tag="biasp", name="bias_t")
        nc.sync.dma_start(out=bias_t[:, :], in_=biasp[:, :])

        def wcol(w, cb, f):       # stationary [128,128]: contract block cb, out block f
            return wt[w][cb][:, f * 128:(f + 1) * 128]

        # live tiles per chunk index
        xt = [None] * NCH
        ht = [None] * NCH
        hsc = [None] * NCH
        xrt = [None] * NCH
        zt = [None] * NCH

        def dma_in(c):
            xt[c] = x_pool.tile([128, 2 * CW], BF16, tag="x", name=f"x_{c}")
            nc.sync.dma_start(out=xt[c][:, :], in_=xC[c])
            ht[c] = h_pool.tile([128, 2 * CW], BF16, tag="h", name=f"h_{c}")
            nc.sync.dma_start(out=ht[c][:, :], in_=hC[c])
            hsc[c] = hs_pool.tile([128, 2 * N_NEIGH * CW], BF16, tag="hs",
                                  name=f"hs_{c}")
            # split hs into two DMAs (neighbors 0-3, 4-7) so the first
            # neighbors' matmuls can start before the full tile lands
            half = N_NEIGH * CW
            nc.sync.dma_start(out=hsc[c][:, 0:half], in_=hsC[c][:, 0:half])
            nc.sync.dma_start(out=hsc[c][:, half:2 * half],
                              in_=hsC[c][:, half:2 * half])

        def mslice(t, cb, ch):    # moving [128,512] of an (f-major) chunk tile
            return t[:, cb * CW + ch * 512:cb * CW + (ch + 1) * 512]

        def hslice(c, n, cb, ch):  # moving [128,512] of neighbor n
            base = (n * 2 + cb) * CW + ch * 512
            return hsc[c][:, base:base + 512]

        def gates_zx(c):
            """z-gate and xr for chunk c: matmuls + ACT drains."""
            pz = ps_pool.tile([128, 2 * CW], F32, tag="ps", name=f"pz_{c}")
            for f in range(2):
                for ch in range(CW // 512):
                    q = pz[:, f * CW + ch * 512:f * CW + (ch + 1) * 512]
                    nc.tensor.matmul(q, wcol("wiz", 0, f), mslice(xt[c], 0, ch),
                                     start=True, stop=False)
                    nc.tensor.matmul(q, wcol("wiz", 1, f), mslice(xt[c], 1, ch),
                                     start=False, stop=False)
                    nc.tensor.matmul(q, wcol("whz", 0, f), mslice(ht[c], 0, ch),
                                     start=False, stop=False)
                    nc.tensor.matmul(q, wcol("whz", 1, f), mslice(ht[c], 1, ch),
                                     start=False, stop=True)
            zt[c] = z_pool.tile([128, 2 * CW], BF16, tag="z", name=f"z_{c}")
            for f in range(2):
                nc.scalar.activation(zt[c][:, f * CW:(f + 1) * CW],
                                     pz[:, f * CW:(f + 1) * CW],
                                     mybir.ActivationFunctionType.Sigmoid,
                                     bias=bias_t[:, f * 3 + 1:f * 3 + 2])

            pxr = ps_pool.tile([128, 2 * CW], F32, tag="ps", name=f"pxr_{c}")
            for f in range(2):
                for ch in range(CW // 512):
                    q = pxr[:, f * CW + ch * 512:f * CW + (ch + 1) * 512]
                    nc.tensor.matmul(q, wcol("wir", 0, f), mslice(xt[c], 0, ch),
                                     start=True, stop=False)
                    nc.tensor.matmul(q, wcol("wir", 1, f), mslice(xt[c], 1, ch),
                                     start=False, stop=True)
            xrt[c] = xr_pool.tile([128, 2 * CW], BF16, tag="xr", name=f"xr_{c}")
            for f in range(2):
                nc.scalar.add(xrt[c][:, f * CW:(f + 1) * CW],
                              pxr[:, f * CW:(f + 1) * CW],
                              bias_t[:, f * 3:f * 3 + 1])

        def neigh(c, n):
            """r_n = sigmoid(Whr@hs_n + xr), product into hsc, running sum."""
            pr = ps_pool.tile([128, 2 * CW], F32, tag="ps", name=f"pr{n}_{c}")
            for f in range(2):
                for ch in range(CW // 512):
                    q = pr[:, f * CW + ch * 512:f * CW + (ch + 1) * 512]
                    nc.tensor.matmul(q, wcol("whr", 0, f), hslice(c, n, 0, ch),
                                     start=True, stop=False)
                    nc.tensor.matmul(q, wcol("whr", 1, f), hslice(c, n, 1, ch),
                                     start=False, stop=False)
                    nc.tensor.matmul(q, id_t[:, :],
                                     mslice(xrt[c], f, ch),
                                     start=False, stop=True)
            rc = r_pool.tile([128, 2 * CW], BF16, tag="r", name=f"r{n}_{c}")
            nc.scalar.activation(rc[:, :], pr[:, :],
                                 mybir.ActivationFunctionType.Sigmoid)
            hs_n = hsc[c][:, n * 2 * CW:(n + 1) * 2 * CW]
            nc.vector.tensor_mul(hs_n, rc[:, :], hs_n)
            if n > 0:
                acc = hsc[c][:, 0:2 * CW]
                with nc.allow_low_precision(reason="bf16 neighbor sums"):
                    nc.vector.tensor_add(acc, acc, hs_n)

        def ngate(c):
            """n gate + combine + output DMA for chunk c."""
            pn = ps_pool.tile([128, 2 * CW], F32, tag="ps", name=f"pn_{c}")
            for f in range(2):
                for ch in range(CW // 512):
                    q = pn[:, f * CW + ch * 512:f * CW + (ch + 1) * 512]
                    nc.tensor.matmul(q, wcol("win", 0, f), mslice(xt[c], 0, ch),
                                     start=True, stop=False)
                    nc.tensor.matmul(q, wcol("win", 1, f), mslice(xt[c], 1, ch),
                                     start=False, stop=False)
                    nc.tensor.matmul(q, wcol("whn", 0, f), mslice(hsc[c], 0, ch),
                                     start=False, stop=False)
                    nc.tensor.matmul(q, wcol("whn", 1, f), mslice(hsc[c], 1, ch),
                                     start=False, stop=True)
            nt = n_pool.tile([128, 2 * CW], BF16, tag="n", name=f"n_{c}")
            for f in range(2):
                nc.scalar.activation(nt[:, f * CW:(f + 1) * CW],
                                     pn[:, f * CW:(f + 1) * CW],
                                     mybir.ActivationFunctionType.Tanh,
                                     bias=bias_t[:, f * 3 + 2:f * 3 + 3])
            # out = n + z * (h - n), bf16 on DVE
            dt_ = d_pool.tile([128, 2 * CW], BF16, tag="d", name=f"d_{c}")
            ot = o_pool.tile([128, 2 * CW], BF16, tag="o", name=f"o_{c}")
            with nc.allow_low_precision(reason="bf16 combine"):
                nc.vector.tensor_sub(dt_[:, :], ht[c][:, :], nt[:, :])
                nc.vector.tensor_mul(dt_[:, :], zt[c][:, :], dt_[:, :])
                nc.vector.tensor_add(ot[:, :], nt[:, :], dt_[:, :])
            nc.sync.dma_start(out=outS[c], in_=ot[:, :])

        # --- software-pipelined emission ---
        dma_in(0)
        gates_zx(0)
        neigh(0, 0)
        neigh(0, 1)
        for c in range(NCH):
            neigh(c, 2)
            neigh(c, 3)
            neigh(c, 4)
            neigh(c, 5)
            if c + 1 < NCH:
                dma_in(c + 1)
                gates_zx(c + 1)
            neigh(c, 6)
            neigh(c, 7)
            if c + 1 < NCH:
                neigh(c + 1, 0)
            ngate(c)
            if c + 1 < NCH:
                neigh(c + 1, 1)

    nc.compile()
    return nc


def _prep_inputs(x, h_sum, hs, Wir, bir, Whr, bhr, Wiz, biz, Whz, bhz,
                 Win, bin_, Whn, bhn):
    """Shard + transpose + chunk-pack to per-core input maps (all bf16)."""
    f32 = np.float32

    def chunk_pack(a):  # [BL, 256] f32 view -> [NCH, 128, 2*CW] bf16
        # out[c, p, f*CW + j] = a[c*CW + j, f*128 + p]
        return np.ascontiguousarray(
            a.reshape(NCH, CW, 2, 128).transpose(0, 3, 2, 1)
        ).reshape(NCH, 128, 2 * CW).astype(BF_NP)

    def hs_pack(a):  # [8, BL, 256] -> [NCH, 128, 16*CW] bf16
        # out[c, p, (n*2+f)*CW + j] = a[n, c*CW + j, f*128 + p]
        return np.ascontiguousarray(
            a.reshape(N_NEIGH, NCH, CW, 2, 128).transpose(1, 4, 0, 3, 2)
        ).reshape(NCH, 128, 2 * N_NEIGH * CW).astype(BF_NP)

    w = {
        "wir": np.ascontiguousarray(np.asarray(Wir, f32).T.astype(BF_NP)),
        "whr": np.ascontiguousarray(np.asarray(Whr, f32).T.astype(BF_NP)),
        "wiz": np.ascontiguousarray(np.asarray(Wiz, f32).T.astype(BF_NP)),
        "whz": np.ascontiguousarray(np.asarray(Whz, f32).T.astype(BF_NP)),
        "win": np.ascontiguousarray(np.asarray(Win, f32).T.astype(BF_NP)),
        "whn": np.ascontiguousarray(np.asarray(Whn, f32).T.astype(BF_NP)),
    }
    b_r = np.asarray(bir, f32) + np.asarray(bhr, f32)
    b_z = np.asarray(biz, f32) + np.asarray(bhz, f32)
    b_n = np.asarray(bin_, f32) + np.asarray(bhn, f32)
    biasp = np.empty((128, 6), f32)
    for f in range(2):
        biasp[:, f * 3 + 0] = b_r[f * 128:(f + 1) * 128]
        biasp[:, f * 3 + 1] = b_z[f * 128:(f + 1) * 128]
        biasp[:, f * 3 + 2] = b_n[f * 128:(f + 1) * 128]
    ident = np.eye(128, dtype=f32).astype(BF_NP)

    xf = np.asarray(x, f32)
    hf = np.asarray(h_sum, f32)
    hsf = np.asarray(hs, f32)

    in_maps = []
    for c in range(M):
        sl = slice(c * BL, (c + 1) * BL)
        m = {
            "xC": chunk_pack(xf[sl]),
            "hC": chunk_pack(hf[sl]),
            "hsC": hs_pack(hsf[:, sl]),
            "ident": ident,
            "biasp": biasp,
        }
        m.update(w)
        in_maps.append(m)
    return in_maps


def _run(inputs, trace=False, **trace_kwargs):
    global _cached
    if _cached is None:
        _cached = _build()
    nc = _cached
    in_maps = _prep_inputs(**inputs)
    res = run_bass_kernel_spmd(nc, in_maps, list(range(M)), trace=trace,
                               **trace_kwargs)
    out = np.empty((B, H), np.float32)
    for c in range(M):
        # outS [NCH, 128, 2*CW] bf16 -> [BL, 256] f32
        o = np.asarray(res.results[c]["outS"]).astype(np.float32)
        o = o.reshape(NCH, 128, 2, CW).transpose(0, 3, 2, 1).reshape(BL, 256)
        out[c * BL:(c + 1) * BL, :] = o
    return out, res


def kernel(**inputs):
    return _run(inputs)[0]


# revision 13
# speedup vs baseline: 1.5079x; 1.0067x over previous
"""GRU-style GNN message-passing kernel for Trainium2 (8 NeuronCores, SPMD).

Reference computation (per node b, features 256, 8 neighbors):
    xr = x @ Wir.T + bir
    hr_n = hs_n @ Whr.T + bhr
    r_n = sigmoid(xr + hr_n)
    z = sigmoid(x @ Wiz.T + biz + h_sum @ Whz.T + bhz)
    s = sum_n r_n * hs_n
    n = tanh(x @ Win.T + bin + s @ Whn.T + bhn)
    out = (1 - z) * n + z * h_sum

Strategy: data-parallel over the node dim B=32768 across 8 cores (4096
rows each), batch-chunked 4x1024 per core. Everything on-chip runs in
feature-major layout [256 features = 2 partition chunks of 128, batch
free dim], all tensors bf16 (fp32 PSUM accumulation). Host pre-packs
each chunk tile contiguously in HBM so every DMA moves 4-32KB/partition
lines. Engine placement per chunk of 1024 nodes:
  - PE: all linear-layer matmuls in [128,512] PSUM-bank groups; the
    shared (xr + b_r) term is injected into each neighbor's PSUM group
    via an identity matmul.
  - ACT: sigmoid/tanh (per-feature bias), xr PSUM drain (+b_r).
  - DVE: r*hs products in place in the hs tile and a linear running sum
    (short dependency tail), plus the final combine out = n + z*(h-n),
    all bf16 2x mode.
Chunk gate-work (z, xr) is emitted one chunk ahead, interleaved into the
neighbor loop, so the two 4-bank PSUM slots rotate without stalling PE.
"""

import sys
import numpy as np
from contextlib import ExitStack

sys.path.insert(0, "/opt/trn_rl_repo")

import ml_dtypes
import concourse.bacc as bacc
import concourse.tile as tile
from concourse import mybir
from concourse.bass_utils import run_bass_kernel_spmd

F32 = mybir.dt.float32
BF16 = mybir.dt.bfloat16
BF_NP = ml_dtypes.bfloat16

N_NEIGH, B, IN, H = 8, 32768, 256, 256
M = 8                    # cores
BL = B // M              # rows per core (4096)
NCH = 4                  # batch chunks per core
CW = BL // NCH           # chunk width (1024)

_cached = None  # compiled program, reused across kernel() calls


def _build():
    nc = bacc.Bacc("TRN2", target_bir_lowering=False, debug=False, num_devices=M)

    xC = nc.dram_tensor("xC", [NCH, 128, 2 * CW], BF16, kind="ExternalInput").ap()
    hC = nc.dram_tensor("hC", [NCH, 128, 2 * CW], BF16, kind="ExternalInput").ap()
    hsC = nc.dram_tensor("hsC", [NCH, 128, 2 * N_NEIGH * CW], BF16,
                         kind="ExternalInput").ap()
    # all bf16 constants in one block: 6 weights x 2 contraction-row blocks
    # of [128,256] each, then the identity [128,128]
    wpack = nc.dram_tensor("wpack", [128, 6 * 512 + 128], BF16,
                           kind="ExternalInput").ap()
    # bias pack: col f*3+j holds feature-chunk f of (b_r, b_z, b_n)[j]
    biasp = nc.dram_tensor("biasp", [128, 6], F32, kind="ExternalInput").ap()
    outS = nc.dram_tensor("outS", [NCH, 128, 2 * CW], BF16,
                          kind="ExternalOutput").ap()

    with tile.TileContext(nc) as tc, ExitStack() as ctx:
        const_pool = ctx.enter_context(tc.tile_pool(name="const", bufs=1))
        x_pool = ctx.enter_context(tc.tile_pool(name="x", bufs=2))
        h_pool = ctx.enter_context(tc.tile_pool(name="h", bufs=2))
        hs_pool = ctx.enter_context(tc.tile_pool(name="hs", bufs=2))
        xr_pool = ctx.enter_context(tc.tile_pool(name="xr", bufs=2))
        z_pool = ctx.enter_context(tc.tile_pool(name="z", bufs=2))
        r_pool = ctx.enter_context(tc.tile_pool(name="r", bufs=4))
        n_pool = ctx.enter_context(tc.tile_pool(name="n", bufs=2))
        d_pool = ctx.enter_context(tc.tile_pool(name="d", bufs=2))
        o_pool = ctx.enter_context(tc.tile_pool(name="o", bufs=2))
        ps_pool = ctx.enter_context(tc.tile_pool(name="ps", bufs=2, space="PSUM"))

        # --- constants: one bf16 block DMA + one small f32 bias DMA ---
        wpk_t = const_pool.tile([128, 6 * 512 + 128], BF16, tag="wpack",
                                name="wpk_t")
        nc.sync.dma_start(out=wpk_t[:, :], in_=wpack[:, :])
        bias_t = const_pool.tile([128, 6], F32, tag="biasp", name="bias_t")
        nc.sync.dma_start(out=bias_t[:, :], in_=biasp[:, :])

        W_ORDER = ("wir", "whr", "wiz", "whz", "win", "whn")

        def wcol(w, cb, f):       # stationary [128,128]: contract block cb, out block f
            # wpack[p, (wi*2+cb)*256 + m] = W.T[cb*128+p, m]
            base = (W_ORDER.index(w) * 2 + cb) * 256 + f * 128
            return wpk_t[:, base:base + 128]

        id_t = wpk_t[:, 6 * 512:6 * 512 + 128]

        # live tiles per chunk index
        xt = [None] * NCH
        ht = [None] * NCH
        hsc = [None] * NCH
        xrt = [None] * NCH
        zt = [None] * NCH

        def dma_in(c, split=2, use_scalar=False):
            xt[c] = x_pool.tile([128, 2 * CW], BF16, tag="x", name=f"x_{c}")
            nc.sync.dma_start(out=xt[c][:, :], in_=xC[c])
            ht[c] = h_pool.tile([128, 2 * CW], BF16, tag="h", name=f"h_{c}")
            (nc.scalar if use_scalar else nc.sync).dma_start(
                out=ht[c][:, :], in_=hC[c])
            hsc[c] = hs_pool.tile([128, 2 * N_NEIGH * CW], BF16, tag="hs",
                                  name=f"hs_{c}")
            # split hs into pieces (chunk 0: spread across sync + scalar
            # queues) so the first neighbors' matmuls start before the full
            # tile lands
            w = 2 * N_NEIGH * CW // split
            for i in range(split):
                eng = nc.scalar if (use_scalar and i % 2 == 1) else nc.sync
                eng.dma_start(out=hsc[c][:, i * w:(i + 1) * w],
                              in_=hsC[c][:, i * w:(i + 1) * w])

        def mslice(t, cb, ch):    # moving [128,512] of an (f-major) chunk tile
            return t[:, cb * CW + ch * 512:cb * CW + (ch + 1) * 512]

        def hslice(c, n, cb, ch):  # moving [128,512] of neighbor n
            base = (n * 2 + cb) * CW + ch * 512
            return hsc[c][:, base:base + 512]

        def gates_zx(c):
            """z-gate and xr for chunk c: matmuls + ACT drains."""
            pz = ps_pool.tile([128, 2 * CW], F32, tag="ps", name=f"pz_{c}")
            for f in range(2):
                for ch in range(CW // 512):
                    q = pz[:, f * CW + ch * 512:f * CW + (ch + 1) * 512]
                    nc.tensor.matmul(q, wcol("wiz", 0, f), mslice(xt[c], 0, ch),
                                     start=True, stop=False)
                    nc.tensor.matmul(q, wcol("wiz", 1, f), mslice(xt[c], 1, ch),
                                     start=False, stop=False)
                    nc.tensor.matmul(q, wcol("whz", 0, f), mslice(ht[c], 0, ch),
                                     start=False, stop=False)
                    nc.tensor.matmul(q, wcol("whz", 1, f), mslice(ht[c], 1, ch),
                                     start=False, stop=True)
            zt[c] = z_pool.tile([128, 2 * CW], BF16, tag="z", name=f"z_{c}")
            for f in range(2):
                nc.scalar.activation(zt[c][:, f * CW:(f + 1) * CW],
                                     pz[:, f * CW:(f + 1) * CW],
                                     mybir.ActivationFunctionType.Sigmoid,
                                     bias=bias_t[:, f * 3 + 1:f * 3 + 2])

            pxr = ps_pool.tile([128, 2 * CW], F32, tag="ps", name=f"pxr_{c}")
            for f in range(2):
                for ch in range(CW // 512):
                    q = pxr[:, f * CW + ch * 512:f * CW + (ch + 1) * 512]
                    nc.tensor.matmul(q, wcol("wir", 0, f), mslice(xt[c], 0, ch),
                                     start=True, stop=False)
                    nc.tensor.matmul(q, wcol("wir", 1, f), mslice(xt[c], 1, ch),
                                     start=False, stop=True)
            xrt[c] = xr_pool.tile([128, 2 * CW], BF16, tag="xr", name=f"xr_{c}")
            for f in range(2):
                nc.scalar.add(xrt[c][:, f * CW:(f + 1) * CW],
                              pxr[:, f * CW:(f + 1) * CW],
                              bias_t[:, f * 3:f * 3 + 1])

        def neigh(c, n):
            """r_n = sigmoid(Whr@hs_n + xr), product into hsc, running sum."""
            pr = ps_pool.tile([128, 2 * CW], F32, tag="ps", name=f"pr{n}_{c}")
            for f in range(2):
                for ch in range(CW // 512):
                    q = pr[:, f * CW + ch * 512:f * CW + (ch + 1) * 512]
                    nc.tensor.matmul(q, wcol("whr", 0, f), hslice(c, n, 0, ch),
                                     start=True, stop=False)
                    nc.tensor.matmul(q, wcol("whr", 1, f), hslice(c, n, 1, ch),
                                     start=False, stop=False)
                    nc.tensor.matmul(q, id_t, mslice(xrt[c], f, ch),
                                     start=False, stop=True)
            rc = r_pool.tile([128, 2 * CW], BF16, tag="r", name=f"r{n}_{c}")
            nc.scalar.activation(rc[:, :], pr[:, :],
                                 mybir.ActivationFunctionType.Sigmoid)
            hs_n = hsc[c][:, n * 2 * CW:(n + 1) * 2 * CW]
            nc.vector.tensor_mul(hs_n, rc[:, :], hs_n)
            if n > 0:
                acc = hsc[c][:, 0:2 * CW]
                with nc.allow_low_precision(reason="bf16 neighbor sums"):
                    nc.vector.tensor_add(acc, acc, hs_n)

        def ngate(c):
            """n gate + combine + output DMA for chunk c."""
            pn = ps_pool.tile([128, 2 * CW], F32, tag="ps", name=f"pn_{c}")
            # all Win matmuls first: they only need x, so PE has work while
            # the DVE accumulation of s finishes
            for f in range(2):
                for ch in range(CW // 512):
                    q = pn[:, f * CW + ch * 512:f * CW + (ch + 1) * 512]
                    nc.tensor.matmul(q, wcol("win", 0, f), mslice(xt[c], 0, ch),
                                     start=True, stop=False)
                    nc.tensor.matmul(q, wcol("win", 1, f), mslice(xt[c], 1, ch),
                                     start=False, stop=False)
            for f in range(2):
                for ch in range(CW // 512):
                    q = pn[:, f * CW + ch * 512:f * CW + (ch + 1) * 512]
                    nc.tensor.matmul(q, wcol("whn", 0, f), mslice(hsc[c], 0, ch),
                                     start=False, stop=False)
                    nc.tensor.matmul(q, wcol("whn", 1, f), mslice(hsc[c], 1, ch),
                                     start=False, stop=True)
            nt = n_pool.tile([128, 2 * CW], BF16, tag="n", name=f"n_{c}")
            dt_ = d_pool.tile([128, 2 * CW], BF16, tag="d", name=f"d_{c}")
            ot = o_pool.tile([128, 2 * CW], BF16, tag="o", name=f"o_{c}")
            # per f-half: tanh then bf16 combine out = n + z*(h - n) on DVE,
            # then its output DMA -- halves the exposed tail latency
            for f in range(2):
                fs = slice(f * CW, (f + 1) * CW)
                nc.scalar.activation(nt[:, fs], pn[:, fs],
                                     mybir.ActivationFunctionType.Tanh,
                                     bias=bias_t[:, f * 3 + 2:f * 3 + 3])
                with nc.allow_low_precision(reason="bf16 combine"):
                    nc.vector.tensor_sub(dt_[:, fs], ht[c][:, fs], nt[:, fs])
                    nc.vector.tensor_mul(dt_[:, fs], zt[c][:, fs], dt_[:, fs])
                    nc.vector.tensor_add(ot[:, fs], nt[:, fs], dt_[:, fs])
                nc.sync.dma_start(out=outS[c][:, fs], in_=ot[:, fs])

        # --- software-pipelined emission ---
        dma_in(0, split=4, use_scalar=True)
        gates_zx(0)
        neigh(0, 0)
        neigh(0, 1)
        for c in range(NCH):
            neigh(c, 2)
            neigh(c, 3)
            neigh(c, 4)
            neigh(c, 5)
            if c + 1 < NCH:
                dma_in(c + 1)
                gates_zx(c + 1)
            neigh(c, 6)
            neigh(c, 7)
            if c + 1 < NCH:
                neigh(c + 1, 0)
                neigh(c + 1, 1)
            ngate(c)

    nc.compile()
    return nc


def _prep_inputs(x, h_sum, hs, Wir, bir, Whr, bhr, Wiz, biz, Whz, bhz,
                 Win, bin_, Whn, bhn):
    """Shard + transpose + chunk-pack to per-core input maps (all bf16)."""
    f32 = np.float32

    def chunk_pack(a):  # [BL, 256] f32 view -> [NCH, 128, 2*CW] bf16
        # out[c, p, f*CW + j] = a[c*CW + j, f*128 + p]
        return np.ascontiguousarray(
            a.reshape(NCH, CW, 2, 128).transpose(0, 3, 2, 1)
        ).reshape(NCH, 128, 2 * CW).astype(BF_NP)

    def hs_pack(a):  # [8, BL, 256] -> [NCH, 128, 16*CW] bf16
        # out[c, p, (n*2+f)*CW + j] = a[n, c*CW + j, f*128 + p]
        return np.ascontiguousarray(
            a.reshape(N_NEIGH, NCH, CW, 2, 128).transpose(1, 4, 0, 3, 2)
        ).reshape(NCH, 128, 2 * N_NEIGH * CW).astype(BF_NP)

    # wpack[p, (wi*2+cb)*256 + m] = W.T[cb*128+p, m]; identity at the end
    wpack = np.empty((128, 6 * 512 + 128), BF_NP)
    for wi, Wm in enumerate((Wir, Whr, Wiz, Whz, Win, Whn)):
        Wt = np.asarray(Wm, f32).T.astype(BF_NP)
        wpack[:, (wi * 2) * 256:(wi * 2 + 1) * 256] = Wt[0:128, :]
        wpack[:, (wi * 2 + 1) * 256:(wi * 2 + 2) * 256] = Wt[128:256, :]
    wpack[:, 6 * 512:] = np.eye(128, dtype=f32).astype(BF_NP)
    b_r = np.asarray(bir, f32) + np.asarray(bhr, f32)
    b_z = np.asarray(biz, f32) + np.asarray(bhz, f32)
    b_n = np.asarray(bin_, f32) + np.asarray(bhn, f32)
    biasp = np.empty((128, 6), f32)
    for f in range(2):
        biasp[:, f * 3 + 0] = b_r[f * 128:(f + 1) * 128]
        biasp[:, f * 3 + 1] = b_z[f * 128:(f + 1) * 128]
        biasp[:, f * 3 + 2] = b_n[f * 128:(f + 1) * 128]

    xf = np.asarray(x, f32)
    hf = np.asarray(h_sum, f32)
    hsf = np.asarray(hs, f32)

    in_maps = []
    for c in range(M):
        sl = slice(c * BL, (c + 1) * BL)
        m = {
            "xC": chunk_pack(xf[sl]),
            "hC": chunk_pack(hf[sl]),
            "hsC": hs_pack(hsf[:, sl]),
            "wpack": wpack,
            "biasp": biasp,
        }
        in_maps.append(m)
    return in_maps


def _run(inputs, trace=False, **trace_kwargs):
    global _cached
    if _cached is None:
        _cached = _build()
    nc = _cached
    in_maps = _prep_inputs(**inputs)
    res = run_bass_kernel_spmd(nc, in_maps, list(range(M)), trace=trace,
                               **trace_kwargs)
    out = np.empty((B, H), np.float32)
    for c in range(M):
        # outS [NCH, 128, 2*CW] bf16 -> [BL, 256] f32
        o = np.asarray(res.results[c]["outS"]).astype(np.float32)
        o = o.reshape(NCH, 128, 2, CW).transpose(0, 3, 2, 1).reshape(BL, 256)
        out[c * BL:(c + 1) * BL, :] = o
    return out, res


def kernel(**inputs):
    return _run(inputs)[0]


# revision 14
# speedup vs baseline: 1.5889x; 1.0537x over previous
"""GRU-style GNN message-passing kernel for Trainium2 (8 NeuronCores, SPMD).

Reference computation (per node b, features 256, 8 neighbors):
    xr = x @ Wir.T + bir
    hr_n = hs_n @ Whr.T + bhr
    r_n = sigmoid(xr + hr_n)
    z = sigmoid(x @ Wiz.T + biz + h_sum @ Whz.T + bhz)
    s = sum_n r_n * hs_n
    n = tanh(x @ Win.T + bin + s @ Whn.T + bhn)
    out = (1 - z) * n + z * h_sum

Strategy: data-parallel over the node dim B=32768 across 8 cores (4096
rows each), batch-chunked 4x1024 per core. Everything on-chip runs in
feature-major layout [256 features = 2 partition chunks of 128, batch
free dim], all tensors bf16 (fp32 PSUM accumulation). Host pre-packs
each chunk tile contiguously in HBM so every DMA moves 4-32KB/partition
lines. Engine placement per chunk of 1024 nodes:
  - PE: all linear-layer matmuls in [128,512] PSUM-bank groups; the
    shared (xr + b_r) term is injected into each neighbor's PSUM group
    via an identity matmul.
  - ACT: sigmoid/tanh (per-feature bias), xr PSUM drain (+b_r).
  - DVE: r*hs products in place in the hs tile and a linear running sum
    (short dependency tail), plus the final combine out = n + z*(h-n),
    all bf16 2x mode.
Chunk gate-work (z, xr) is emitted one chunk ahead, interleaved into the
neighbor loop, so the two 4-bank PSUM slots rotate without stalling PE.
"""

import sys
import numpy as np
from contextlib import ExitStack

sys.path.insert(0, "/opt/trn_rl_repo")

import ml_dtypes
import concourse.bacc as bacc
import concourse.tile as tile
from concourse import mybir
from concourse.bass_utils import run_bass_kernel_spmd

F32 = mybir.dt.float32
BF16 = mybir.dt.bfloat16
BF_NP = ml_dtypes.bfloat16

N_NEIGH, B, IN, H = 8, 32768, 256, 256
M = 8                    # cores
BL = B // M              # rows per core (4096)
NCH = 4                  # batch chunks per core
CW = BL // NCH           # chunk width (1024)

_cached = None  # compiled program, reused across kernel() calls


def _build():
    nc = bacc.Bacc("TRN2", target_bir_lowering=False, debug=False, num_devices=M)

    xC = nc.dram_tensor("xC", [NCH, 128, 2 * CW], BF16, kind="ExternalInput").ap()
    hC = nc.dram_tensor("hC", [NCH, 128, 2 * CW], BF16, kind="ExternalInput").ap()
    hsC = nc.dram_tensor("hsC", [NCH, 128, 2 * N_NEIGH * CW], BF16,
                         kind="ExternalInput").ap()
    # all bf16 constants in one block: 6 weights x 2 contraction-row blocks
    # of [128,256] each, then the identity [128,128]
    wpack = nc.dram_tensor("wpack", [128, 6 * 512 + 128], BF16,
                           kind="ExternalInput").ap()
    # bias pack: col f*3+j holds feature-chunk f of (b_r, b_z, b_n)[j]
    biasp = nc.dram_tensor("biasp", [128, 6], F32, kind="ExternalInput").ap()
    outS = nc.dram_tensor("outS", [NCH, 128, 2 * CW], BF16,
                          kind="ExternalOutput").ap()

    with tile.TileContext(nc) as tc, ExitStack() as ctx:
        const_pool = ctx.enter_context(tc.tile_pool(name="const", bufs=1))
        x_pool = ctx.enter_context(tc.tile_pool(name="x", bufs=2))
        h_pool = ctx.enter_context(tc.tile_pool(name="h", bufs=2))
        hs_pool = ctx.enter_context(tc.tile_pool(name="hs", bufs=2))
        xr_pool = ctx.enter_context(tc.tile_pool(name="xr", bufs=2))
        z_pool = ctx.enter_context(tc.tile_pool(name="z", bufs=2))
        r_pool = ctx.enter_context(tc.tile_pool(name="r", bufs=4))
        n_pool = ctx.enter_context(tc.tile_pool(name="n", bufs=2))
        d_pool = ctx.enter_context(tc.tile_pool(name="d", bufs=2))
        o_pool = ctx.enter_context(tc.tile_pool(name="o", bufs=2))
        ps_pool = ctx.enter_context(tc.tile_pool(name="ps", bufs=2, space="PSUM"))

        # --- constants: one bf16 block DMA + one small f32 bias DMA ---
        wpk_t = const_pool.tile([128, 6 * 512 + 128], BF16, tag="wpack",
                                name="wpk_t")
        nc.sync.dma_start(out=wpk_t[:, :], in_=wpack[:, :])
        bias_t = const_pool.tile([128, 6], F32, tag="biasp", name="bias_t")
        nc.sync.dma_start(out=bias_t[:, :], in_=biasp[:, :])

        W_ORDER = ("wir", "whr", "wiz", "whz", "win", "whn")

        def wcol(w, cb, f):       # stationary [128,128]: contract block cb, out block f
            # wpack[p, (wi*2+cb)*256 + m] = W.T[cb*128+p, m]
            base = (W_ORDER.index(w) * 2 + cb) * 256 + f * 128
            return wpk_t[:, base:base + 128]

        id_t = wpk_t[:, 6 * 512:6 * 512 + 128]

        # live tiles per chunk index
        xt = [None] * NCH
        ht = [None] * NCH
        hsc = [None] * NCH
        xrt = [None] * NCH
        zt = [None] * NCH

        def dma_in(c, split=2, use_scalar=False):
            xt[c] = x_pool.tile([128, 2 * CW], BF16, tag="x", name=f"x_{c}")
            nc.sync.dma_start(out=xt[c][:, :], in_=xC[c])
            ht[c] = h_pool.tile([128, 2 * CW], BF16, tag="h", name=f"h_{c}")
            (nc.scalar if use_scalar else nc.sync).dma_start(
                out=ht[c][:, :], in_=hC[c])
            hsc[c] = hs_pool.tile([128, 2 * N_NEIGH * CW], BF16, tag="hs",
                                  name=f"hs_{c}")
            # split hs into pieces (chunk 0: spread across sync + scalar
            # queues) so the first neighbors' matmuls start before the full
            # tile lands
            w = 2 * N_NEIGH * CW // split
            for i in range(split):
                eng = nc.scalar if (use_scalar and i % 2 == 1) else nc.sync
                eng.dma_start(out=hsc[c][:, i * w:(i + 1) * w],
                              in_=hsC[c][:, i * w:(i + 1) * w])

        def mslice(t, cb, ch):    # moving [128,512] of an (f-major) chunk tile
            return t[:, cb * CW + ch * 512:cb * CW + (ch + 1) * 512]

        def hslice(c, n, cb, ch):  # moving [128,512] of neighbor n
            base = (n * 2 + cb) * CW + ch * 512
            return hsc[c][:, base:base + 512]

        def gates_zx(c):
            """z-gate and xr for chunk c: matmuls + ACT drains."""
            pz = ps_pool.tile([128, 2 * CW], F32, tag="ps", name=f"pz_{c}")
            for f in range(2):
                for ch in range(CW // 512):
                    q = pz[:, f * CW + ch * 512:f * CW + (ch + 1) * 512]
                    nc.tensor.matmul(q, wcol("wiz", 0, f), mslice(xt[c], 0, ch),
                                     start=True, stop=False)
                    nc.tensor.matmul(q, wcol("wiz", 1, f), mslice(xt[c], 1, ch),
                                     start=False, stop=False)
                    nc.tensor.matmul(q, wcol("whz", 0, f), mslice(ht[c], 0, ch),
                                     start=False, stop=False)
                    nc.tensor.matmul(q, wcol("whz", 1, f), mslice(ht[c], 1, ch),
                                     start=False, stop=True)
            zt[c] = z_pool.tile([128, 2 * CW], BF16, tag="z", name=f"z_{c}")
            for f in range(2):
                nc.scalar.activation(zt[c][:, f * CW:(f + 1) * CW],
                                     pz[:, f * CW:(f + 1) * CW],
                                     mybir.ActivationFunctionType.Sigmoid,
                                     bias=bias_t[:, f * 3 + 1:f * 3 + 2])

            pxr = ps_pool.tile([128, 2 * CW], F32, tag="ps", name=f"pxr_{c}")
            for f in range(2):
                for ch in range(CW // 512):
                    q = pxr[:, f * CW + ch * 512:f * CW + (ch + 1) * 512]
                    nc.tensor.matmul(q, wcol("wir", 0, f), mslice(xt[c], 0, ch),
                                     start=True, stop=False)
                    nc.tensor.matmul(q, wcol("wir", 1, f), mslice(xt[c], 1, ch),
                                     start=False, stop=True)
            xrt[c] = xr_pool.tile([128, 2 * CW], BF16, tag="xr", name=f"xr_{c}")
            for f in range(2):
                nc.scalar.add(xrt[c][:, f * CW:(f + 1) * CW],
                              pxr[:, f * CW:(f + 1) * CW],
                              bias_t[:, f * 3:f * 3 + 1])

        def neigh(c, n):
            """r_n = sigmoid(Whr@hs_n + xr), product into hsc, running sum."""
            pr = ps_pool.tile([128, 2 * CW], F32, tag="ps", name=f"pr{n}_{c}")
            for f in range(2):
                for ch in range(CW // 512):
                    q = pr[:, f * CW + ch * 512:f * CW + (ch + 1) * 512]
                    nc.tensor.matmul(q, wcol("whr", 0, f), hslice(c, n, 0, ch),
                                     start=True, stop=False)
                    nc.tensor.matmul(q, wcol("whr", 1, f), hslice(c, n, 1, ch),
                                     start=False, stop=False)
                    nc.tensor.matmul(q, id_t, mslice(xrt[c], f, ch),
                                     start=False, stop=True)
            rc = r_pool.tile([128, 2 * CW], BF16, tag="r", name=f"r{n}_{c}")
            nc.scalar.activation(rc[:, :], pr[:, :],
                                 mybir.ActivationFunctionType.Sigmoid)
            hs_n = hsc[c][:, n * 2 * CW:(n + 1) * 2 * CW]
            nc.vector.tensor_mul(hs_n, rc[:, :], hs_n)
            if n > 0:
                acc = hsc[c][:, 0:2 * CW]
                with nc.allow_low_precision(reason="bf16 neighbor sums"):
                    nc.vector.tensor_add(acc, acc, hs_n)

        def ngate(c):
            """n gate + combine + output DMA for chunk c."""
            pn = ps_pool.tile([128, 2 * CW], F32, tag="ps", name=f"pn_{c}")
            # all Win matmuls first: they only need x, so PE has work while
            # the DVE accumulation of s finishes
            for f in range(2):
                for ch in range(CW // 512):
                    q = pn[:, f * CW + ch * 512:f * CW + (ch + 1) * 512]
                    nc.tensor.matmul(q, wcol("win", 0, f), mslice(xt[c], 0, ch),
                                     start=True, stop=False)
                    nc.tensor.matmul(q, wcol("win", 1, f), mslice(xt[c], 1, ch),
                                     start=False, stop=False)
            for f in range(2):
                for ch in range(CW // 512):
                    q = pn[:, f * CW + ch * 512:f * CW + (ch + 1) * 512]
                    nc.tensor.matmul(q, wcol("whn", 0, f), mslice(hsc[c], 0, ch),
                                     start=False, stop=False)
                    nc.tensor.matmul(q, wcol("whn", 1, f), mslice(hsc[c], 1, ch),
                                     start=False, stop=True)
            nt = n_pool.tile([128, 2 * CW], BF16, tag="n", name=f"n_{c}")
            dt_ = d_pool.tile([128, 2 * CW], BF16, tag="d", name=f"d_{c}")
            ot = o_pool.tile([128, 2 * CW], BF16, tag="o", name=f"o_{c}")
            # per f-half: tanh then bf16 combine out = n + z*(h - n) on DVE,
            # then its output DMA -- halves the exposed tail latency
            for f in range(2):
                fs = slice(f * CW, (f + 1) * CW)
                nc.scalar.activation(nt[:, fs], pn[:, fs],
                                     mybir.ActivationFunctionType.Tanh,
                                     bias=bias_t[:, f * 3 + 2:f * 3 + 3])
                with nc.allow_low_precision(reason="bf16 combine"):
                    nc.vector.tensor_sub(dt_[:, fs], ht[c][:, fs], nt[:, fs])
                    nc.vector.tensor_mul(dt_[:, fs], zt[c][:, fs], dt_[:, fs])
                    nc.vector.tensor_add(ot[:, fs], nt[:, fs], dt_[:, fs])
                nc.sync.dma_start(out=outS[c][:, fs], in_=ot[:, fs])

        # --- software-pipelined emission ---
        dma_in(0, split=4, use_scalar=True)
        gates_zx(0)
        neigh(0, 0)
        neigh(0, 1)
        for c in range(NCH):
            neigh(c, 2)
            neigh(c, 3)
            neigh(c, 4)
            neigh(c, 5)
            if c + 1 < NCH:
                dma_in(c + 1)
                gates_zx(c + 1)
            neigh(c, 6)
            neigh(c, 7)
            if c + 1 < NCH:
                neigh(c + 1, 0)
            ngate(c)
            if c + 1 < NCH:
                neigh(c + 1, 1)

    nc.compile()
    return nc


def _prep_inputs(x, h_sum, hs, Wir, bir, Whr, bhr, Wiz, biz, Whz, bhz,
                 Win, bin_, Whn, bhn):
    """Shard + transpose + chunk-pack to per-core input maps (all bf16)."""
    f32 = np.float32

    def chunk_pack(a):  # [BL, 256] f32 view -> [NCH, 128, 2*CW] bf16
        # out[c, p, f*CW + j] = a[c*CW + j, f*128 + p]
        return np.ascontiguousarray(
            a.reshape(NCH, CW, 2, 128).transpose(0, 3, 2, 1)
        ).reshape(NCH, 128, 2 * CW).astype(BF_NP)

    def hs_pack(a):  # [8, BL, 256] -> [NCH, 128, 16*CW] bf16
        # out[c, p, (n*2+f)*CW + j] = a[n, c*CW + j, f*128 + p]
        return np.ascontiguousarray(
            a.reshape(N_NEIGH, NCH, CW, 2, 128).transpose(1, 4, 0, 3, 2)
        ).reshape(NCH, 128, 2 * N_NEIGH * CW).astype(BF_NP)

    # wpack[p, (wi*2+cb)*256 + m] = W.T[cb*128+p, m]; identity at the end
    wpack = np.empty((128, 6 * 512 + 128), BF_NP)
    for wi, Wm in enumerate((Wir, Whr, Wiz, Whz, Win, Whn)):
        Wt = np.asarray(Wm, f32).T.astype(BF_NP)
        wpack[:, (wi * 2) * 256:(wi * 2 + 1) * 256] = Wt[0:128, :]
        wpack[:, (wi * 2 + 1) * 256:(wi * 2 + 2) * 256] = Wt[128:256, :]
    wpack[:, 6 * 512:] = np.eye(128, dtype=f32).astype(BF_NP)
    b_r = np.asarray(bir, f32) + np.asarray(bhr, f32)
    b_z = np.asarray(biz, f32) + np.asarray(bhz, f32)
    b_n = np.asarray(bin_, f32) + np.asarray(bhn, f32)
    biasp = np.empty((128, 6), f32)
    for f in range(2):
        biasp[:, f * 3 + 0] = b_r[f * 128:(f + 1) * 128]
        biasp[:, f * 3 + 1] = b_z[f * 128:(f + 1) * 128]
        biasp[:, f * 3 + 2] = b_n[f * 128:(f + 1) * 128]

    xf = np.asarray(x, f32)
    hf = np.asarray(h_sum, f32)
    hsf = np.asarray(hs, f32)

    in_maps = []
    for c in range(M):
        sl = slice(c * BL, (c + 1) * BL)
        m = {
            "xC": chunk_pack(xf[sl]),
            "hC": chunk_pack(hf[sl]),
            "hsC": hs_pack(hsf[:, sl]),
            "wpack": wpack,
            "biasp": biasp,
        }
        in_maps.append(m)
    return in_maps


def _run(inputs, trace=False, **trace_kwargs):
    global _cached
    if _cached is None:
        _cached = _build()
    nc = _cached
    in_maps = _prep_inputs(**inputs)
    res = run_bass_kernel_spmd(nc, in_maps, list(range(M)), trace=trace,
                               **trace_kwargs)
    out = np.empty((B, H), np.float32)
    for c in range(M):
        # outS [NCH, 128, 2*CW] bf16 -> [BL, 256] f32
        o = np.asarray(res.results[c]["outS"]).astype(np.float32)
        o = o.reshape(NCH, 128, 2, CW).transpose(0, 3, 2, 1).reshape(BL, 256)
        out[c * BL:(c + 1) * BL, :] = o
    return out, res


def kernel(**inputs):
    return _run(inputs)[0]
